# revision 30
# baseline (speedup 1.0000x reference)
"""Multi-head self-attention (B=4,S=2048,D=1024,H=16,DH=64, causal) on 8 trn2 cores.

Sharding: core c -> batch b=c//2, head-group g=c%2 (8 heads each).

v4 (458us v1 -> 310us v2 -> this):
 - ScalarE runs ONLY the softmax exps (one fused ACT per head-pair per key
   block over a [128, 2x512] 2-bank PSUM tile, diagonal blocks trimmed).
   Biases folded: scale*bq rides the Q PSUM->SBUF copy as a per-partition
   scalar; bk dropped exactly (softmax shift invariance); bv/bp become a
   host-side constant (attn rows sum to 1).
 - t-loop software-pipelined: AV matmuls lag scores/exp by one step, so the
   next pair's scores+exp cover the renorm latency of the previous pair.
 - Renorm: two quick PSUM->SBUF copies (den row to a partition-0 tile for
   reciprocal_approx_fast, av+den block for the multiply) free the PSUM
   accumulator in ~1.3us; causal-mask multiplies run on GpSimd so the DVE
   renorm burst never blocks them.
 - PE head-of-line waits on exp are filled with one interleaved matmul per
   key block: QKV chains of the NEXT s-block (must drain by block end) and
   out-projection of the PREVIOUS block (carries over freely).
 - Score-pair matmuls adjacent -> row-group (64/64) concurrent on the array.
 - b=0's attention is hand-interleaved into the upfront QKV(0) phase.

K-projection quirk (reference views k as (B,S,DH,H)): head h uses Wk rows
[dh*16+h for dh in range(64)] -- handled by host-side row gather.
"""
import numpy as np

import concourse.mybir as mybir
import concourse.tile as tile
from concourse import bacc
from concourse.bass_utils import run_bass_kernel_spmd

F32 = mybir.dt.float32
BF16 = mybir.dt.bfloat16
AF = mybir.ActivationFunctionType
MUL = mybir.AluOpType.mult

B, S, D, H, DH = 4, 2048, 1024, 16, 64
FG = 512          # features per head-group (8 heads * 64)
N_CORES = 8
SCALE = 0.125     # 1/sqrt(64)

_NC = None


def _build():
    nc = bacc.Bacc("TRN2", target_bir_lowering=False, debug=False,
                   num_devices=N_CORES, enable_asserts=False)
    xT_d = nc.dram_tensor("xT", [D, S], F32, kind="ExternalInput").ap()
    wqT_d = nc.dram_tensor("wqT", [D, FG], F32, kind="ExternalInput").ap()
    wkT_d = nc.dram_tensor("wkT", [D, FG], F32, kind="ExternalInput").ap()
    wvT_d = nc.dram_tensor("wvT", [D, FG], F32, kind="ExternalInput").ap()
    wpT_d = nc.dram_tensor("wpT", [FG, D], F32, kind="ExternalInput").ap()
    bqs_d = nc.dram_tensor("bqs", [128, 4], F32, kind="ExternalInput").ap()
    msk_d = nc.dram_tensor("msk", [128, 2, 128], F32, kind="ExternalInput").ap()
    out_d = nc.dram_tensor("outT", [D, S], F32, kind="ExternalOutput").ap()

    with tile.TileContext(nc) as tc:
        with tc.tile_pool(name="persist", bufs=1) as pp, \
             tc.tile_pool(name="xin", bufs=2) as xp, \
             tc.tile_pool(name="etile", bufs=10) as ep, \
             tc.tile_pool(name="small", bufs=4) as sp, \
             tc.tile_pool(name="avdp", bufs=4) as avp, \
             tc.tile_pool(name="outtile", bufs=3) as op, \
             tc.tile_pool(name="wstage", bufs=2) as wsp, \
             tc.tile_pool(name="psprs", bufs=2, space="PSUM") as ps_s, \
             tc.tile_pool(name="psoth", bufs=2, space="PSUM") as ps_o:

            # ---- persistent SBUF tensors ----
            wq = pp.tile([128, 8, FG], BF16)   # [dp, do, f]
            wk = pp.tile([128, 8, FG], BF16)
            wv = pp.tile([128, 8, FG], BF16)
            wp = pp.tile([128, 4, D], BF16)    # [cp, co, j]
            qt = pp.tile([128, 4, S], BF16)    # [fp, fo, s]
            kt = pp.tile([128, 4, S], BF16)
            va = pp.tile([128, 16, 8, DH + 1], BF16)  # [skp, sko, h, dh|1]
            on_ = pp.tile([128, 4, S], BF16)   # renormed out^T  [cp, co, s]
            msk = pp.tile([128, 2, 128], BF16)
            bqs = pp.tile([128, 4], F32)

            # casting DMAs must use the gpsimd queue.  wq rides the otherwise
            # idle sync queue as raw f32 (cast on DVE) so the Q chains start
            # ~8us in instead of waiting for serialized gpsimd transfers.
            # startup-critical tensors ride BOTH queues as raw f32 (cast-DMAs
            # run at ~1/3 bandwidth) and get cast on the otherwise idle DVE
            wqT_r = wqT_d.rearrange("(do dp) f -> dp do f", dp=128)
            xT_r = xT_d.rearrange("(do dp) s -> dp do s", dp=128)
            nc.sync.dma_start(bqs[:], bqs_d[:])
            xblk0 = xp.tile([128, 8, 512], BF16, tag="x")
            x0s = wsp.tile([128, 8, 512], F32, tag="stg")
            nc.sync.dma_start(x0s[:], xT_r[:, :, 0:512])
            nc.vector.tensor_copy(xblk0[:], x0s[:])
            wqs = wsp.tile([128, 8, FG], F32, tag="stg")
            nc.sync.dma_start(wqs[:, :, 0:128], wqT_r[:, :, 0:128])
            nc.vector.tensor_copy(wq[:, :, 0:128], wqs[:, :, 0:128])
            nc.sync.dma_start(wqs[:, :, 128:FG], wqT_r[:, :, 128:FG])
            nc.vector.tensor_copy(wq[:, :, 128:FG], wqs[:, :, 128:FG])
            nc.vector.memset(va[:, :, :, DH:DH + 1], 1.0)

            # ---------- direct chain emitters ----------
            def emit_qk_chain(sb, xblk, which, ft):
                wt, dst, bias = ((wq, qt, bqs) if which == 'q' else (wk, kt, None))
                psq = ps_o.tile([128, 512], F32, name="psq", space="PSUM", tag="mm")
                for do in range(8):
                    nc.tensor.matmul(psq[:], wt[:, do, ft * 128:(ft + 1) * 128],
                                     xblk[:, do, :], start=(do == 0), stop=(do == 7))
                d = dst[:, ft, sb * 512:(sb + 1) * 512]
                if bias is not None:
                    nc.vector.tensor_scalar_add(d, psq[:], bias[:, ft:ft + 1])
                else:
                    nc.vector.tensor_copy(d, psq[:])

            def emit_v_chain(sb, xblk, stt):
                psv = ps_o.tile([128, 512], F32, name="psv", space="PSUM", tag="mm")
                for do in range(8):
                    nc.tensor.matmul(psv[:], xblk[:, do, stt * 128:(stt + 1) * 128],
                                     wv[:, do, :], start=(do == 0), stop=(do == 7))
                nc.vector.tensor_copy(va[:, sb * 4 + stt, :, :DH],
                                      psv[:].rearrange("p (h d) -> p h d", h=8))

            # ---------- filler step queues ----------
            qkv_q, proj_q = [], []
            qpos = [0]

            def qkv_steps(sb, xblk):
                for which in ('q', 'k'):
                    for ft in range(4):
                        st = {}
                        for do in range(8):
                            def th(which=which, ft=ft, do=do, st=st, sb=sb,
                                   xblk=xblk):
                                wt, dst, bias = ((wq, qt, bqs) if which == 'q'
                                                 else (wk, kt, None))
                                if do == 0:
                                    st['ps'] = ps_o.tile([128, 512], F32,
                                                         name="psq",
                                                         space="PSUM", tag="mm")
                                nc.tensor.matmul(
                                    st['ps'][:], wt[:, do, ft * 128:(ft + 1) * 128],
                                    xblk[:, do, :], start=(do == 0), stop=(do == 7))
                                if do == 7:
                                    d = dst[:, ft, sb * 512:(sb + 1) * 512]
                                    if bias is not None:
                                        nc.vector.tensor_scalar_add(
                                            d, st['ps'][:], bias[:, ft:ft + 1])
                                    else:
                                        nc.vector.tensor_copy(d, st['ps'][:])
                            qkv_q.append(th)
                for stt in range(4):
                    st = {}
                    for do in range(8):
                        def th(stt=stt, do=do, st=st, sb=sb, xblk=xblk):
                            if do == 0:
                                st['ps'] = ps_o.tile([128, 512], F32, name="psv",
                                                     space="PSUM", tag="mm")
                            nc.tensor.matmul(
                                st['ps'][:], xblk[:, do, stt * 128:(stt + 1) * 128],
                                wv[:, do, :], start=(do == 0), stop=(do == 7))
                            if do == 7:
                                nc.vector.tensor_copy(
                                    va[:, sb * 4 + stt, :, :DH],
                                    st['ps'][:].rearrange("p (h d) -> p h d", h=8))
                        qkv_q.append(th)

            def proj_steps(bb, jts):
                for jt in jts:
                    st = {}
                    for co in range(4):
                        def th(jt=jt, co=co, st=st, bb=bb):
                            if co == 0:
                                st['ps'] = ps_o.tile([128, 512], F32, name="psj",
                                                     space="PSUM", tag="mm")
                            nc.tensor.matmul(
                                st['ps'][:], wp[:, co, jt * 128:(jt + 1) * 128],
                                on_[:, co, bb * 512:(bb + 1) * 512],
                                start=(co == 0), stop=(co == 3))
                            if co == 3:
                                osb = op.tile([128, 512], F32, tag="o")
                                nc.vector.tensor_copy(osb[:], st['ps'][:])
                                nc.sync.dma_start(
                                    out_d[jt * 128:(jt + 1) * 128,
                                          bb * 512:(bb + 1) * 512], osb[:])
                        proj_q.append(th)

            def filler(n=1):
                for _ in range(n):
                    if qpos[0] < len(qkv_q):
                        qkv_q[qpos[0]]()
                        qpos[0] += 1
                    elif proj_q:
                        proj_q.pop(0)()

            def drain_qkv():
                while qpos[0] < len(qkv_q):
                    qkv_q[qpos[0]]()
                    qpos[0] += 1

            # ---------- attention pieces ----------
            def emit_scores(b, p, t):
                m = t - 4 * b
                c0 = 128 * m if m > 0 else 0
                ksl = slice(t * 128, (t + 1) * 128)
                qsl = slice(b * 512 + c0, (b + 1) * 512)
                spr = ps_s.tile([128, 2, 512], F32, name="spr", space="PSUM",
                                tag="s")
                nc.tensor.matmul(spr[:, 0, c0:], kt[0:64, p, ksl],
                                 qt[0:64, p, qsl], start=True, stop=True)
                nc.tensor.matmul(spr[:, 1, c0:], kt[64:128, p, ksl],
                                 qt[64:128, p, qsl], start=True, stop=True)
                e = ep.tile([128, 2, 512], BF16, tag="e")
                nc.scalar.activation(e[:, :, c0:], spr[:, :, c0:], AF.Exp)
                if m >= 0:  # causal mask on the diagonal 128-strip
                    nc.vector.tensor_tensor(e[:, :, c0:c0 + 128],
                                            e[:, :, c0:c0 + 128], msk[:], MUL)
                return e, c0

            def emit_av(b, p, t, e, c0, ot0, ot1, nt):
                nc.tensor.matmul(ot0[:, c0:], va[:, t, 2 * p, :], e[:, 0, c0:],
                                 start=(t == 0), stop=(t == nt - 1),
                                 skip_group_check=True)
                nc.tensor.matmul(ot1[:, c0:], va[:, t, 2 * p + 1, :],
                                 e[:, 1, c0:],
                                 start=(t == 0), stop=(t == nt - 1),
                                 skip_group_check=True)

            def emit_renorm(b, p, ot0, ot1):
                # part A (now): free the PSUM accumulators with two quick
                # copies.  part B (returned thunks): reciprocal + broadcast +
                # multiply, deferred into the next pair's early steps so the
                # DVE burst never delays that pair's mask/AV chain.
                fins = []
                for h, otp in ((2 * p, ot0), (2 * p + 1, ot1)):
                    den = sp.tile([1, 512], F32, tag="den")
                    nc.vector.tensor_copy(den[:], otp[DH:DH + 1, :])
                    avd = avp.tile([DH, 512], F32, tag="avd")
                    nc.vector.tensor_copy(avd[:], otp[0:DH, :])  # frees bank

                    def fin(h=h, den=den, avd=avd, b=b, p=p):
                        rec = sp.tile([1, 512], F32, tag="rec")
                        nc.vector.reciprocal_approx_fast(rec[:], den[:])
                        rb = sp.tile([DH, 512], F32, tag="rb")
                        nc.gpsimd.partition_broadcast(rb[:], rec[:])
                        r0 = 64 * (h % 2)
                        dst = on_[r0:r0 + 64, p, b * 512:(b + 1) * 512]
                        nc.vector.tensor_tensor(dst, avd[:], rb[:], MUL)
                    fins.append(fin)
                return fins

            def attn_pair(b, p, nt, pend_fins, fpt=2):
                ot0 = ps_o.tile([DH + 1, 512], F32, name="ot0", space="PSUM",
                                tag="ot")
                ot1 = ps_o.tile([DH + 1, 512], F32, name="ot1", space="PSUM",
                                tag="ot")
                pend = None
                for t in range(nt):
                    # fillers/finishers FIRST: PE never head-of-line-blocks on
                    # the spair buffer, and AV(t-1) finds its exp long done
                    if pend_fins:
                        pend_fins.pop(0)()
                    filler(fpt)
                    e, c0 = emit_scores(b, p, t)
                    if pend is not None:
                        emit_av(b, p, t - 1, pend[0], pend[1], ot0, ot1, nt)
                    pend = (e, c0)
                filler(fpt)
                emit_av(b, p, nt - 1, pend[0], pend[1], ot0, ot1, nt)
                return emit_renorm(b, p, ot0, ot1)

            # ---------- b = 0: attention interleaved into upfront QKV ----------
            nc.gpsimd.dma_start(msk[:], msk_d[:])
            wvs = wsp.tile([128, 8, FG], F32, tag="stg")
            nc.gpsimd.dma_start(wvs[:], wvT_d.rearrange("(do dp) f -> dp do f", dp=128))
            nc.vector.tensor_copy(wv[:], wvs[:])
            wks = wsp.tile([128, 8, FG], F32, tag="stg")
            nc.sync.dma_start(wks[:], wkT_d.rearrange("(do dp) f -> dp do f", dp=128))
            nc.vector.tensor_copy(wk[:], wks[:])

            xblk1 = xp.tile([128, 8, 512], BF16, tag="x")
            nc.gpsimd.dma_start(xblk1[:], xT_r[:, :, 512:1024])
            nc.gpsimd.dma_start(wp[:], wpT_d.rearrange("(co cp) j -> cp co j", cp=128))
            qkv_steps(1, xblk1)          # filler supply during b=0

            # pipelined b=0 schedule: scores/exps of pair p overlap the QK
            # chains of pair p+1 and the AV loops of earlier pairs
            es_all = {}

            def b0_scores(p):
                es_all[p] = [emit_scores(0, p, t) for t in range(4)]

            def b0_avloop(p, fins):
                ot0 = ps_o.tile([DH + 1, 512], F32, name="ot0", space="PSUM",
                                tag="ot")
                ot1 = ps_o.tile([DH + 1, 512], F32, name="ot1", space="PSUM",
                                tag="ot")
                for t in range(4):
                    if fins:
                        fins.pop(0)()
                    filler(1)
                    emit_av(0, p, t, es_all[p][t][0], es_all[p][t][1],
                            ot0, ot1, 4)
                del es_all[p]
                return emit_renorm(0, p, ot0, ot1)

            emit_qk_chain(0, xblk0, 'q', 0)
            emit_qk_chain(0, xblk0, 'k', 0)
            b0_scores(0)
            emit_qk_chain(0, xblk0, 'q', 1)
            emit_qk_chain(0, xblk0, 'k', 1)
            b0_scores(1)
            for stt in range(4):
                emit_v_chain(0, xblk0, stt)
            fins = b0_avloop(0, [])
            emit_qk_chain(0, xblk0, 'q', 2)
            emit_qk_chain(0, xblk0, 'k', 2)
            b0_scores(2)
            fins = b0_avloop(1, fins)
            emit_qk_chain(0, xblk0, 'q', 3)
            emit_qk_chain(0, xblk0, 'k', 3)
            b0_scores(3)
            fins = b0_avloop(2, fins)
            fins = b0_avloop(3, fins)
            drain_qkv()

            # ---------- b = 1..3 ----------
            for b in range(1, 4):
                proj_steps(b - 1, range(8))
                if b < 3:
                    xblkn = xp.tile([128, 8, 512], BF16, tag="x")
                    nc.gpsimd.dma_start(
                        xblkn[:], xT_r[:, :, (b + 1) * 512:(b + 2) * 512])
                    qkv_steps(b + 1, xblkn)
                nt = 4 * b + 4
                for p in range(4):
                    fins = attn_pair(b, p, nt, fins, fpt=1)
                drain_qkv()
            for f in fins:
                f()
            while proj_q:
                proj_q.pop(0)()
            proj_steps(3, range(8))
            while proj_q:
                proj_q.pop(0)()

    nc.compile()
    return nc


def kernel(x, Wq, bq, Wk, bk, Wv, bv, Wp, bp):
    global _NC
    if _NC is None:
        _NC = _build()

    x = np.asarray(x, np.float32)
    Wq, bq = np.asarray(Wq, np.float32), np.asarray(bq, np.float32)
    Wk, bk = np.asarray(Wk, np.float32), np.asarray(bk, np.float32)
    Wv, bv = np.asarray(Wv, np.float32), np.asarray(bv, np.float32)
    Wp, bp = np.asarray(Wp, np.float32), np.asarray(bp, np.float32)

    # diagonal-strip causal mask, duplicated for the two heads of a pair
    i = np.arange(128)[:, None]
    j = np.arange(128)[None, :]
    mstrip = (i <= j).astype(np.float32)            # [128, 128]
    msk = np.broadcast_to(mstrip[:, None, :], (128, 2, 128)).copy()

    # host-folded constant: attn rows sum to 1 -> out += bv, then @Wp.T
    host_bias = Wp @ bv + bp                        # [D]

    in_maps = []
    for c in range(N_CORES):
        b, g = c // 2, c % 2
        hs = range(8 * g, 8 * g + 8)
        kidx = np.array([dh * 16 + h for h in hs for dh in range(DH)])
        fsl = slice(FG * g, FG * (g + 1))
        in_maps.append({
            "xT": np.ascontiguousarray(x[b].T),
            "wqT": np.ascontiguousarray((SCALE * Wq[fsl]).T),
            "wkT": np.ascontiguousarray(Wk[kidx].T),
            "wvT": np.ascontiguousarray(Wv[fsl].T),
            "wpT": np.ascontiguousarray(Wp[:, fsl].T),
            "bqs": np.ascontiguousarray((SCALE * bq[fsl]).reshape(4, 128).T),
            "msk": msk,
        })

    res = run_bass_kernel_spmd(_NC, in_maps, core_ids=list(range(N_CORES)))
    out = np.empty((B, S, D), np.float32)
    for b in range(B):
        acc = res.results[2 * b]["outT"] + res.results[2 * b + 1]["outT"]
        out[b] = acc.T + host_bias
    return out


# revision 31
# speedup vs baseline: 1.0379x; 1.0379x over previous
"""Multi-head self-attention (B=4,S=2048,D=1024,H=16,DH=64, causal) on 8 trn2 cores.

Sharding: core c -> batch b=c//2, head-group g=c%2 (8 heads each).

v4 (458us v1 -> 310us v2 -> this):
 - ScalarE runs ONLY the softmax exps (one fused ACT per head-pair per key
   block over a [128, 2x512] 2-bank PSUM tile, diagonal blocks trimmed).
   Biases folded: scale*bq rides the Q PSUM->SBUF copy as a per-partition
   scalar; bk dropped exactly (softmax shift invariance); bv/bp become a
   host-side constant (attn rows sum to 1).
 - t-loop software-pipelined: AV matmuls lag scores/exp by one step, so the
   next pair's scores+exp cover the renorm latency of the previous pair.
 - Renorm: two quick PSUM->SBUF copies (den row to a partition-0 tile for
   reciprocal_approx_fast, av+den block for the multiply) free the PSUM
   accumulator in ~1.3us; causal-mask multiplies run on GpSimd so the DVE
   renorm burst never blocks them.
 - PE head-of-line waits on exp are filled with one interleaved matmul per
   key block: QKV chains of the NEXT s-block (must drain by block end) and
   out-projection of the PREVIOUS block (carries over freely).
 - Score-pair matmuls adjacent -> row-group (64/64) concurrent on the array.
 - b=0's attention is hand-interleaved into the upfront QKV(0) phase.

K-projection quirk (reference views k as (B,S,DH,H)): head h uses Wk rows
[dh*16+h for dh in range(64)] -- handled by host-side row gather.
"""
import numpy as np

import concourse.mybir as mybir
import concourse.tile as tile
from concourse import bacc
from concourse.bass_utils import run_bass_kernel_spmd

F32 = mybir.dt.float32
BF16 = mybir.dt.bfloat16
AF = mybir.ActivationFunctionType
MUL = mybir.AluOpType.mult

B, S, D, H, DH = 4, 2048, 1024, 16, 64
FG = 512          # features per head-group (8 heads * 64)
N_CORES = 8
SCALE = 0.125     # 1/sqrt(64)

_NC = None


def _build():
    nc = bacc.Bacc("TRN2", target_bir_lowering=False, debug=False,
                   num_devices=N_CORES, enable_asserts=False)
    xT_d = nc.dram_tensor("xT", [128, 8, S], F32, kind="ExternalInput").ap()
    wqT_d = nc.dram_tensor("wqT", [128, 8, FG], F32, kind="ExternalInput").ap()
    wkT_d = nc.dram_tensor("wkT", [128, 8, FG], F32, kind="ExternalInput").ap()
    wvT_d = nc.dram_tensor("wvT", [128, 8, FG], F32, kind="ExternalInput").ap()
    wpT_d = nc.dram_tensor("wpT", [128, 4, D], F32, kind="ExternalInput").ap()
    bqs_d = nc.dram_tensor("bqs", [128, 4], F32, kind="ExternalInput").ap()
    msk_d = nc.dram_tensor("msk", [128, 2, 128], F32, kind="ExternalInput").ap()
    out_d = nc.dram_tensor("outT", [D, S], F32, kind="ExternalOutput").ap()

    with tile.TileContext(nc) as tc:
        with tc.tile_pool(name="persist", bufs=1) as pp, \
             tc.tile_pool(name="xin", bufs=2) as xp, \
             tc.tile_pool(name="etile", bufs=10) as ep, \
             tc.tile_pool(name="small", bufs=4) as sp, \
             tc.tile_pool(name="avdp", bufs=4) as avp, \
             tc.tile_pool(name="outtile", bufs=3) as op, \
             tc.tile_pool(name="wstage", bufs=2) as wsp, \
             tc.tile_pool(name="psprs", bufs=2, space="PSUM") as ps_s, \
             tc.tile_pool(name="psoth", bufs=2, space="PSUM") as ps_o:

            # ---- persistent SBUF tensors ----
            wq = pp.tile([128, 8, FG], BF16)   # [dp, do, f]
            wk = pp.tile([128, 8, FG], BF16)
            wv = pp.tile([128, 8, FG], BF16)
            wp = pp.tile([128, 4, D], BF16)    # [cp, co, j]
            qt = pp.tile([128, 4, S], BF16)    # [fp, fo, s]
            kt = pp.tile([128, 4, S], BF16)
            va = pp.tile([128, 16, 8, DH + 1], BF16)  # [skp, sko, h, dh|1]
            on_ = pp.tile([128, 4, S], BF16)   # renormed out^T  [cp, co, s]
            msk = pp.tile([128, 2, 128], BF16)
            bqs = pp.tile([128, 4], F32)

            # casting DMAs must use the gpsimd queue.  wq rides the otherwise
            # idle sync queue as raw f32 (cast on DVE) so the Q chains start
            # ~8us in instead of waiting for serialized gpsimd transfers.
            # inputs are HOST-pre-arranged to the on-chip layout, so every
            # DMA moves large contiguous per-partition runs.  x0/wv ride the
            # sync queue as raw f32 (cast on idle DVE); casting DMAs (gpsimd-
            # only) carry the rest.
            xT_r = xT_d
            nc.sync.dma_start(bqs[:], bqs_d[:])
            xblk0 = xp.tile([128, 8, 512], BF16, tag="x")
            x0s = wsp.tile([128, 8, 512], F32, tag="stg")
            nc.sync.dma_start(x0s[:], xT_r[:, :, 0:512])
            nc.vector.tensor_copy(xblk0[:], x0s[:])
            nc.gpsimd.dma_start(msk[:], msk_d[:])
            nc.gpsimd.dma_start(wq[:], wqT_d[:])
            nc.gpsimd.dma_start(wk[:], wkT_d[:])
            nc.vector.memset(va[:, :, :, DH:DH + 1], 1.0)

            # ---------- direct chain emitters ----------
            def emit_qk_chain(sb, xblk, which, ft):
                wt, dst, bias = ((wq, qt, bqs) if which == 'q' else (wk, kt, None))
                psq = ps_o.tile([128, 512], F32, name="psq", space="PSUM", tag="mm")
                for do in range(8):
                    nc.tensor.matmul(psq[:], wt[:, do, ft * 128:(ft + 1) * 128],
                                     xblk[:, do, :], start=(do == 0), stop=(do == 7))
                d = dst[:, ft, sb * 512:(sb + 1) * 512]
                if bias is not None:
                    nc.vector.tensor_scalar_add(d, psq[:], bias[:, ft:ft + 1])
                else:
                    nc.vector.tensor_copy(d, psq[:])

            def emit_v_chain(sb, xblk, stt):
                psv = ps_o.tile([128, 512], F32, name="psv", space="PSUM", tag="mm")
                for do in range(8):
                    nc.tensor.matmul(psv[:], xblk[:, do, stt * 128:(stt + 1) * 128],
                                     wv[:, do, :], start=(do == 0), stop=(do == 7))
                nc.vector.tensor_copy(va[:, sb * 4 + stt, :, :DH],
                                      psv[:].rearrange("p (h d) -> p h d", h=8))

            # ---------- filler step queues ----------
            qkv_q, proj_q = [], []
            qpos = [0]

            def qkv_steps(sb, xblk):
                for which in ('q', 'k'):
                    for ft in range(4):
                        st = {}
                        for do in range(8):
                            def th(which=which, ft=ft, do=do, st=st, sb=sb,
                                   xblk=xblk):
                                wt, dst, bias = ((wq, qt, bqs) if which == 'q'
                                                 else (wk, kt, None))
                                if do == 0:
                                    st['ps'] = ps_o.tile([128, 512], F32,
                                                         name="psq",
                                                         space="PSUM", tag="mm")
                                nc.tensor.matmul(
                                    st['ps'][:], wt[:, do, ft * 128:(ft + 1) * 128],
                                    xblk[:, do, :], start=(do == 0), stop=(do == 7))
                                if do == 7:
                                    d = dst[:, ft, sb * 512:(sb + 1) * 512]
                                    if bias is not None:
                                        nc.vector.tensor_scalar_add(
                                            d, st['ps'][:], bias[:, ft:ft + 1])
                                    else:
                                        nc.vector.tensor_copy(d, st['ps'][:])
                            qkv_q.append(th)
                for stt in range(4):
                    st = {}
                    for do in range(8):
                        def th(stt=stt, do=do, st=st, sb=sb, xblk=xblk):
                            if do == 0:
                                st['ps'] = ps_o.tile([128, 512], F32, name="psv",
                                                     space="PSUM", tag="mm")
                            nc.tensor.matmul(
                                st['ps'][:], xblk[:, do, stt * 128:(stt + 1) * 128],
                                wv[:, do, :], start=(do == 0), stop=(do == 7))
                            if do == 7:
                                nc.vector.tensor_copy(
                                    va[:, sb * 4 + stt, :, :DH],
                                    st['ps'][:].rearrange("p (h d) -> p h d", h=8))
                        qkv_q.append(th)

            def proj_steps(bb, jts):
                for jt in jts:
                    st = {}
                    for co in range(4):
                        def th(jt=jt, co=co, st=st, bb=bb):
                            if co == 0:
                                st['ps'] = ps_o.tile([128, 512], F32, name="psj",
                                                     space="PSUM", tag="mm")
                            nc.tensor.matmul(
                                st['ps'][:], wp[:, co, jt * 128:(jt + 1) * 128],
                                on_[:, co, bb * 512:(bb + 1) * 512],
                                start=(co == 0), stop=(co == 3))
                            if co == 3:
                                osb = op.tile([128, 512], F32, tag="o")
                                nc.vector.tensor_copy(osb[:], st['ps'][:])
                                nc.sync.dma_start(
                                    out_d[jt * 128:(jt + 1) * 128,
                                          bb * 512:(bb + 1) * 512], osb[:])
                        proj_q.append(th)

            def filler(n=1):
                for _ in range(n):
                    if qpos[0] < len(qkv_q):
                        qkv_q[qpos[0]]()
                        qpos[0] += 1
                    elif proj_q:
                        proj_q.pop(0)()

            def drain_qkv():
                while qpos[0] < len(qkv_q):
                    qkv_q[qpos[0]]()
                    qpos[0] += 1

            # ---------- attention pieces ----------
            def emit_scores(b, p, t):
                m = t - 4 * b
                c0 = 128 * m if m > 0 else 0
                ksl = slice(t * 128, (t + 1) * 128)
                qsl = slice(b * 512 + c0, (b + 1) * 512)
                spr = ps_s.tile([128, 2, 512], F32, name="spr", space="PSUM",
                                tag="s")
                nc.tensor.matmul(spr[:, 0, c0:], kt[0:64, p, ksl],
                                 qt[0:64, p, qsl], start=True, stop=True)
                nc.tensor.matmul(spr[:, 1, c0:], kt[64:128, p, ksl],
                                 qt[64:128, p, qsl], start=True, stop=True)
                e = ep.tile([128, 2, 512], BF16, tag="e")
                nc.scalar.activation(e[:, :, c0:], spr[:, :, c0:], AF.Exp)
                if m >= 0:  # causal mask on the diagonal 128-strip
                    nc.vector.tensor_tensor(e[:, :, c0:c0 + 128],
                                            e[:, :, c0:c0 + 128], msk[:], MUL)
                return e, c0

            def emit_av(b, p, t, e, c0, ot0, ot1, nt):
                nc.tensor.matmul(ot0[:, c0:], va[:, t, 2 * p, :], e[:, 0, c0:],
                                 start=(t == 0), stop=(t == nt - 1),
                                 skip_group_check=True)
                nc.tensor.matmul(ot1[:, c0:], va[:, t, 2 * p + 1, :],
                                 e[:, 1, c0:],
                                 start=(t == 0), stop=(t == nt - 1),
                                 skip_group_check=True)

            def emit_renorm(b, p, ot0, ot1):
                # part A (now): free the PSUM accumulators with two quick
                # copies.  part B (returned thunks): reciprocal + broadcast +
                # multiply, deferred into the next pair's early steps so the
                # DVE burst never delays that pair's mask/AV chain.
                fins = []
                for h, otp in ((2 * p, ot0), (2 * p + 1, ot1)):
                    den = sp.tile([1, 512], F32, tag="den")
                    nc.vector.tensor_copy(den[:], otp[DH:DH + 1, :])
                    avd = avp.tile([DH, 512], F32, tag="avd")
                    nc.vector.tensor_copy(avd[:], otp[0:DH, :])  # frees bank

                    def fin(h=h, den=den, avd=avd, b=b, p=p):
                        rec = sp.tile([1, 512], F32, tag="rec")
                        nc.vector.reciprocal_approx_fast(rec[:], den[:])
                        rb = sp.tile([DH, 512], F32, tag="rb")
                        nc.gpsimd.partition_broadcast(rb[:], rec[:])
                        r0 = 64 * (h % 2)
                        dst = on_[r0:r0 + 64, p, b * 512:(b + 1) * 512]
                        nc.vector.tensor_tensor(dst, avd[:], rb[:], MUL)
                    fins.append(fin)
                return fins

            def attn_pair(b, p, nt, pend_fins, fpt=2):
                ot0 = ps_o.tile([DH + 1, 512], F32, name="ot0", space="PSUM",
                                tag="ot")
                ot1 = ps_o.tile([DH + 1, 512], F32, name="ot1", space="PSUM",
                                tag="ot")
                pend = None
                for t in range(nt):
                    # fillers/finishers FIRST: PE never head-of-line-blocks on
                    # the spair buffer, and AV(t-1) finds its exp long done
                    if pend_fins:
                        pend_fins.pop(0)()
                    filler(fpt)
                    e, c0 = emit_scores(b, p, t)
                    if pend is not None:
                        emit_av(b, p, t - 1, pend[0], pend[1], ot0, ot1, nt)
                    pend = (e, c0)
                filler(fpt)
                emit_av(b, p, nt - 1, pend[0], pend[1], ot0, ot1, nt)
                return emit_renorm(b, p, ot0, ot1)

            # ---------- b = 0: attention interleaved into upfront QKV ----------
            wvs = wsp.tile([128, 8, FG], F32, tag="stg")
            nc.sync.dma_start(wvs[:], wvT_d[:])
            nc.vector.tensor_copy(wv[:], wvs[:])

            xblk1 = xp.tile([128, 8, 512], BF16, tag="x")
            nc.gpsimd.dma_start(xblk1[:], xT_r[:, :, 512:1024])
            nc.gpsimd.dma_start(wp[:], wpT_d[:])
            qkv_steps(1, xblk1)          # filler supply during b=0

            # pipelined b=0 schedule: scores/exps of pair p overlap the QK
            # chains of pair p+1 and the AV loops of earlier pairs
            es_all = {}

            def b0_scores(p):
                es_all[p] = [emit_scores(0, p, t) for t in range(4)]

            def b0_avloop(p, fins):
                ot0 = ps_o.tile([DH + 1, 512], F32, name="ot0", space="PSUM",
                                tag="ot")
                ot1 = ps_o.tile([DH + 1, 512], F32, name="ot1", space="PSUM",
                                tag="ot")
                for t in range(4):
                    if fins:
                        fins.pop(0)()
                    filler(1)
                    emit_av(0, p, t, es_all[p][t][0], es_all[p][t][1],
                            ot0, ot1, 4)
                del es_all[p]
                return emit_renorm(0, p, ot0, ot1)

            emit_qk_chain(0, xblk0, 'q', 0)
            emit_qk_chain(0, xblk0, 'k', 0)
            b0_scores(0)
            emit_qk_chain(0, xblk0, 'q', 1)
            emit_qk_chain(0, xblk0, 'k', 1)
            b0_scores(1)
            for stt in range(4):
                emit_v_chain(0, xblk0, stt)
            fins = b0_avloop(0, [])
            emit_qk_chain(0, xblk0, 'q', 2)
            emit_qk_chain(0, xblk0, 'k', 2)
            b0_scores(2)
            fins = b0_avloop(1, fins)
            emit_qk_chain(0, xblk0, 'q', 3)
            emit_qk_chain(0, xblk0, 'k', 3)
            b0_scores(3)
            fins = b0_avloop(2, fins)
            fins = b0_avloop(3, fins)
            drain_qkv()

            # ---------- b = 1..3 ----------
            for b in range(1, 4):
                proj_steps(b - 1, range(8))
                if b < 3:
                    xblkn = xp.tile([128, 8, 512], BF16, tag="x")
                    nc.gpsimd.dma_start(
                        xblkn[:], xT_r[:, :, (b + 1) * 512:(b + 2) * 512])
                    qkv_steps(b + 1, xblkn)
                nt = 4 * b + 4
                for p in range(4):
                    fins = attn_pair(b, p, nt, fins, fpt=1)
                drain_qkv()
            for f in fins:
                f()
            while proj_q:
                proj_q.pop(0)()
            proj_steps(3, range(8))
            while proj_q:
                proj_q.pop(0)()

    nc.compile()
    return nc


def kernel(x, Wq, bq, Wk, bk, Wv, bv, Wp, bp):
    global _NC
    if _NC is None:
        _NC = _build()

    x = np.asarray(x, np.float32)
    Wq, bq = np.asarray(Wq, np.float32), np.asarray(bq, np.float32)
    Wk, bk = np.asarray(Wk, np.float32), np.asarray(bk, np.float32)
    Wv, bv = np.asarray(Wv, np.float32), np.asarray(bv, np.float32)
    Wp, bp = np.asarray(Wp, np.float32), np.asarray(bp, np.float32)

    # diagonal-strip causal mask, duplicated for the two heads of a pair
    i = np.arange(128)[:, None]
    j = np.arange(128)[None, :]
    mstrip = (i <= j).astype(np.float32)            # [128, 128]
    msk = np.broadcast_to(mstrip[:, None, :], (128, 2, 128)).copy()

    # host-folded constant: attn rows sum to 1 -> out += bv, then @Wp.T
    host_bias = Wp @ bv + bp                        # [D]

    in_maps = []
    for c in range(N_CORES):
        b, g = c // 2, c % 2
        hs = range(8 * g, 8 * g + 8)
        kidx = np.array([dh * 16 + h for h in hs for dh in range(DH)])
        fsl = slice(FG * g, FG * (g + 1))
        in_maps.append({
            "xT": np.ascontiguousarray(
                x[b].T.reshape(8, 128, S).transpose(1, 0, 2)),
            "wqT": np.ascontiguousarray(
                (SCALE * Wq[fsl]).T.reshape(8, 128, FG).transpose(1, 0, 2)),
            "wkT": np.ascontiguousarray(
                Wk[kidx].T.reshape(8, 128, FG).transpose(1, 0, 2)),
            "wvT": np.ascontiguousarray(
                Wv[fsl].T.reshape(8, 128, FG).transpose(1, 0, 2)),
            "wpT": np.ascontiguousarray(
                Wp[:, fsl].T.reshape(4, 128, D).transpose(1, 0, 2)),
            "bqs": np.ascontiguousarray((SCALE * bq[fsl]).reshape(4, 128).T),
            "msk": msk,
        })

    res = run_bass_kernel_spmd(_NC, in_maps, core_ids=list(range(N_CORES)))
    out = np.empty((B, S, D), np.float32)
    for b in range(B):
        acc = res.results[2 * b]["outT"] + res.results[2 * b + 1]["outT"]
        out[b] = acc.T + host_bias
    return out
